# revision 1
# baseline (speedup 1.0000x reference)
import numpy as np
import jax
import jax.numpy as jnp
from jax import lax

# Hardcoded problem dims (nn_Attention_43155831390219)
DIM = 256
HEADS = 8
SR = 4
WS = 7
B = 16
HH = 56
WW = 56
N = HH * WW
NCORES = 8


def _layernorm(x, g, b, eps=1e-5):
    mu = jnp.mean(x, -1, keepdims=True)
    var = jnp.mean(jnp.square(x - mu), -1, keepdims=True)
    return (x - mu) * lax.rsqrt(var + eps) * g + b


def _win_part(t, H, W, ws):
    b, h, _, d = t.shape
    t = t.reshape(b, h, H // ws, ws, W // ws, ws, d)
    t = jnp.transpose(t, (0, 1, 2, 4, 3, 5, 6))
    return t.reshape(b, h, (H // ws) * (W // ws), ws * ws, d)


def _forward_shard(x, lepe_lin_w, lepe_lin_b, lepe_conv_w, lepe_conv_b,
                   sr_w, sr_b, ln_g, ln_b, q1_w, kv1_w, q2_w, kv2_w,
                   proj_w, proj_b):
    H, W = HH, WW
    Bb, Nn, C = x.shape
    h2 = HEADS // 2
    hd = C // HEADS
    scale = hd ** -0.5

    # lepe: linear -> depthwise 3x3 conv
    t = x @ lepe_lin_w + lepe_lin_b
    img = jnp.transpose(t, (0, 2, 1)).reshape(Bb, C, H, W)
    lepe = lax.conv_general_dilated(img, lepe_conv_w, (1, 1), ((1, 1), (1, 1)),
                                    dimension_numbers=("NCHW", "OIHW", "NCHW"),
                                    feature_group_count=C) + lepe_conv_b[None, :, None, None]
    lepe = jnp.transpose(lepe.reshape(Bb, C, Nn), (0, 2, 1))

    # ---- global branch (sr-downsampled KV) ----
    q1 = jnp.transpose((x @ q1_w).reshape(Bb, Nn, h2, hd), (0, 2, 1, 3))
    x_img = jnp.transpose(x, (0, 2, 1)).reshape(Bb, C, H, W)
    xs = lax.conv_general_dilated(x_img, sr_w, (SR, SR), "VALID",
                                  dimension_numbers=("NCHW", "OIHW", "NCHW")) + sr_b[None, :, None, None]
    Nk = (H // SR) * (W // SR)
    xs = jnp.transpose(xs.reshape(Bb, C, Nk), (0, 2, 1))
    xs = jax.nn.gelu(_layernorm(xs, ln_g, ln_b), approximate=False)
    kv1 = (xs @ kv1_w).reshape(Bb, Nk, 2, h2, hd)
    k1 = jnp.transpose(kv1[:, :, 0], (0, 2, 1, 3))
    v1 = jnp.transpose(kv1[:, :, 1], (0, 2, 1, 3))
    attn1 = jax.nn.softmax(jnp.einsum("bhnd,bhmd->bhnm", q1, k1) * scale, axis=-1)
    x1 = jnp.einsum("bhnm,bhmd->bhnd", attn1, v1)
    x1 = jnp.transpose(x1, (0, 2, 1, 3)).reshape(Bb, Nn, C // 2)

    gm = jnp.mean(lax.stop_gradient(attn1), axis=(1, 2))
    gm = gm.reshape(Bb, H // SR, W // SR)
    gm = jnp.repeat(jnp.repeat(gm, SR, axis=1), SR, axis=2)

    # ---- local branch (7x7 windowed) ----
    q2 = jnp.transpose((x @ q2_w).reshape(Bb, Nn, h2, hd), (0, 2, 1, 3))
    kv2 = (x @ kv2_w).reshape(Bb, Nn, 2, h2, hd)
    k2 = jnp.transpose(kv2[:, :, 0], (0, 2, 1, 3))
    v2 = jnp.transpose(kv2[:, :, 1], (0, 2, 1, 3))
    q2w = _win_part(q2, H, W, WS)
    k2w = _win_part(k2, H, W, WS)
    v2w = _win_part(v2, H, W, WS)
    attn2 = jax.nn.softmax(jnp.einsum("bhwqd,bhwkd->bhwqk", q2w, k2w) * scale, axis=-1)
    x2w = jnp.einsum("bhwqk,bhwkd->bhwqd", attn2, v2w)
    x2 = x2w.reshape(Bb, h2, H // WS, W // WS, WS, WS, hd)
    x2 = jnp.transpose(x2, (0, 2, 4, 3, 5, 1, 6)).reshape(Bb, Nn, h2 * hd)

    lm = jnp.mean(lax.stop_gradient(attn2), axis=(1, 3))
    lm = lm.reshape(Bb, H // WS, W // WS, WS, WS)
    lm = jnp.transpose(lm, (0, 1, 3, 2, 4)).reshape(Bb, H, W)

    out = jnp.concatenate([x1, x2], axis=-1)
    out = (out + lepe) @ proj_w + proj_b

    mask = lm + gm
    mask_1 = mask.reshape(Bb, H * W)
    mask_2 = jnp.transpose(mask, (0, 2, 1)).reshape(Bb, H * W)
    return out, mask_1, mask_2


_PMAPPED = None


def _get_pmapped():
    global _PMAPPED
    if _PMAPPED is None:
        devs = jax.devices()[:NCORES]
        _PMAPPED = jax.pmap(
            _forward_shard,
            devices=devs,
            in_axes=(0,) + (None,) * 14,
        )
    return _PMAPPED


def kernel(x, H, W, lepe_lin_w, lepe_lin_b, lepe_conv_w, lepe_conv_b,
           sr_w, sr_b, ln_g, ln_b, q1_w, kv1_w, q2_w, kv2_w, proj_w, proj_b):
    # Data-parallel over batch B=16 across 8 NeuronCores (2 per core).
    x = np.ascontiguousarray(np.asarray(x, dtype=np.float32))
    per = B // NCORES
    xs = x.reshape(NCORES, per, N, DIM)
    ws = [np.asarray(w, dtype=np.float32) for w in (
        lepe_lin_w, lepe_lin_b, lepe_conv_w, lepe_conv_b,
        sr_w, sr_b, ln_g, ln_b, q1_w, kv1_w, q2_w, kv2_w, proj_w, proj_b)]
    with jax.default_matmul_precision("highest"):
        out, m1, m2 = _get_pmapped()(xs, *ws)
    out = np.asarray(out).reshape(B, N, DIM).astype(np.float32)
    m1 = np.asarray(m1).reshape(B, N).astype(np.float32)
    m2 = np.asarray(m2).reshape(B, N).astype(np.float32)
    return out, m1, m2


# revision 2
# speedup vs baseline: 1.0049x; 1.0049x over previous
import numpy as np
import jax
import jax.numpy as jnp
from jax import lax

# Hardcoded problem dims (nn_Attention_43155831390219)
DIM = 256
HEADS = 8
SR = 4
WS = 7
B = 16
HH = 56
WW = 56
N = HH * WW
NCORES = 8


def _layernorm(x, g, b, eps=1e-5):
    mu = jnp.mean(x, -1, keepdims=True)
    var = jnp.mean(jnp.square(x - mu), -1, keepdims=True)
    return (x - mu) * lax.rsqrt(var + eps) * g + b


def _win_part(t, H, W, ws):
    b, h, _, d = t.shape
    t = t.reshape(b, h, H // ws, ws, W // ws, ws, d)
    t = jnp.transpose(t, (0, 1, 2, 4, 3, 5, 6))
    return t.reshape(b, h, (H // ws) * (W // ws), ws * ws, d)


def _forward_shard(x, lepe_lin_w, lepe_lin_b, lepe_conv_w, lepe_conv_b,
                   sr_w, sr_b, ln_g, ln_b, q1_w, kv1_w, q2_w, kv2_w,
                   proj_w, proj_b):
    H, W = HH, WW
    Bb, Nn, C = x.shape
    h2 = HEADS // 2
    hd = C // HEADS
    scale = hd ** -0.5

    # lepe: linear -> depthwise 3x3 conv
    t = x @ lepe_lin_w + lepe_lin_b
    img = jnp.transpose(t, (0, 2, 1)).reshape(Bb, C, H, W)
    lepe = lax.conv_general_dilated(img, lepe_conv_w, (1, 1), ((1, 1), (1, 1)),
                                    dimension_numbers=("NCHW", "OIHW", "NCHW"),
                                    feature_group_count=C) + lepe_conv_b[None, :, None, None]
    lepe = jnp.transpose(lepe.reshape(Bb, C, Nn), (0, 2, 1))

    # ---- global branch (sr-downsampled KV) ----
    q1 = jnp.transpose((x @ q1_w).reshape(Bb, Nn, h2, hd), (0, 2, 1, 3))
    x_img = jnp.transpose(x, (0, 2, 1)).reshape(Bb, C, H, W)
    xs = lax.conv_general_dilated(x_img, sr_w, (SR, SR), "VALID",
                                  dimension_numbers=("NCHW", "OIHW", "NCHW")) + sr_b[None, :, None, None]
    Nk = (H // SR) * (W // SR)
    xs = jnp.transpose(xs.reshape(Bb, C, Nk), (0, 2, 1))
    xs = jax.nn.gelu(_layernorm(xs, ln_g, ln_b), approximate=False)
    kv1 = (xs @ kv1_w).reshape(Bb, Nk, 2, h2, hd)
    k1 = jnp.transpose(kv1[:, :, 0], (0, 2, 1, 3))
    v1 = jnp.transpose(kv1[:, :, 1], (0, 2, 1, 3))
    attn1 = jax.nn.softmax(jnp.einsum("bhnd,bhmd->bhnm", q1, k1) * scale, axis=-1)
    x1 = jnp.einsum("bhnm,bhmd->bhnd", attn1, v1)
    x1 = jnp.transpose(x1, (0, 2, 1, 3)).reshape(Bb, Nn, C // 2)

    gm = jnp.mean(lax.stop_gradient(attn1), axis=(1, 2))
    gm = gm.reshape(Bb, H // SR, 1, W // SR, 1)
    gm = jnp.broadcast_to(gm, (Bb, H // SR, SR, W // SR, SR)).reshape(Bb, H, W)

    # ---- local branch (7x7 windowed) ----
    q2 = jnp.transpose((x @ q2_w).reshape(Bb, Nn, h2, hd), (0, 2, 1, 3))
    kv2 = (x @ kv2_w).reshape(Bb, Nn, 2, h2, hd)
    k2 = jnp.transpose(kv2[:, :, 0], (0, 2, 1, 3))
    v2 = jnp.transpose(kv2[:, :, 1], (0, 2, 1, 3))
    q2w = _win_part(q2, H, W, WS)
    k2w = _win_part(k2, H, W, WS)
    v2w = _win_part(v2, H, W, WS)
    attn2 = jax.nn.softmax(jnp.einsum("bhwqd,bhwkd->bhwqk", q2w, k2w) * scale, axis=-1)
    x2w = jnp.einsum("bhwqk,bhwkd->bhwqd", attn2, v2w)
    x2 = x2w.reshape(Bb, h2, H // WS, W // WS, WS, WS, hd)
    x2 = jnp.transpose(x2, (0, 2, 4, 3, 5, 1, 6)).reshape(Bb, Nn, h2 * hd)

    lm = jnp.mean(lax.stop_gradient(attn2), axis=(1, 3))
    lm = lm.reshape(Bb, H // WS, W // WS, WS, WS)
    lm = jnp.transpose(lm, (0, 1, 3, 2, 4)).reshape(Bb, H, W)

    out = jnp.concatenate([x1, x2], axis=-1)
    out = (out + lepe) @ proj_w + proj_b

    mask = lm + gm
    mask_1 = mask.reshape(Bb, H * W)
    mask_2 = jnp.transpose(mask, (0, 2, 1)).reshape(Bb, H * W)
    return out, mask_1, mask_2


_PMAPPED = None


def _get_pmapped():
    global _PMAPPED
    if _PMAPPED is None:
        devs = jax.devices()[:NCORES]
        _PMAPPED = jax.pmap(
            _forward_shard,
            devices=devs,
            in_axes=(0,) + (None,) * 14,
        )
    return _PMAPPED


def kernel(x, H, W, lepe_lin_w, lepe_lin_b, lepe_conv_w, lepe_conv_b,
           sr_w, sr_b, ln_g, ln_b, q1_w, kv1_w, q2_w, kv2_w, proj_w, proj_b):
    # Data-parallel over batch B=16 across 8 NeuronCores (2 per core).
    x = np.ascontiguousarray(np.asarray(x, dtype=np.float32))
    per = B // NCORES
    xs = x.reshape(NCORES, per, N, DIM)
    ws = [np.asarray(w, dtype=np.float32) for w in (
        lepe_lin_w, lepe_lin_b, lepe_conv_w, lepe_conv_b,
        sr_w, sr_b, ln_g, ln_b, q1_w, kv1_w, q2_w, kv2_w, proj_w, proj_b)]
    with jax.default_matmul_precision("highest"):
        out, m1, m2 = _get_pmapped()(xs, *ws)
    out = np.asarray(out).reshape(B, N, DIM).astype(np.float32)
    m1 = np.asarray(m1).reshape(B, N).astype(np.float32)
    m2 = np.asarray(m2).reshape(B, N).astype(np.float32)
    return out, m1, m2
